# revision 2
# baseline (speedup 1.0000x reference)
# Trainium2 Bass kernel for nn_DEERLIFNode (DEER fixed-point LIF neuron).
#
# Math: the reference runs MAX_ITER=10 damped-Newton (DEER) iterations
#   ys = shift(y); h = ys + (x - ys)/TAU; G = -(decay*(1 - sg))
#   y  = solve(y[t] = -G[t]*y[t-1] + h[t] + G[t]*ys[t])
# At the fixed point ys[t] = y[t-1], so the G terms cancel exactly and the
# fixed point satisfies y[t] = h[t] = y[t-1] + (x[t] - y[t-1])/TAU, i.e. the
# plain leaky integrator y[t] = 0.5*y[t-1] + 0.5*x[t] (TAU=2).  The iteration
# contracts ~3.3x per step, so after 10 iterations the reference output IS the
# fixed point to ~2e-5 relative (measured in f64: y rel err 2.1e-5, 17 spike
# flips out of 16.7M => spike rel err 3.0e-3, both far under the 2e-2 gate).
#
# So the kernel is a single linear scan per (b, f) lane.  Scaled form avoids
# preprocessing x:  w[t] = 0.5*w[t-1] + x[t], w[-1] = 2*v0  (so w = 2*y):
#   y     = 0.5*w                (exact power-of-2 scale)
#   spike = (w >= 1.4)           (fp32(1.4) == 2*fp32(0.7) exactly, so this
#                                 matches the reference threshold bit-for-bit)
#
# Layout: lanes = (b, f) pairs on SBUF partitions, time on the free axis.
# Each of the 8 cores takes 2048 lanes = 16 partition-tiles of [128, 1024].
# DRAM buffers are laid out [128, NTILES*T] so every DMA is a plain column
# slice with >=2KiB contiguous runs per partition (full DMA bandwidth).
#
# Per tile-pair (2 tiles batched per DMA to halve HWDGE pressure):
#   DMA in : x pair [128, 2048] fp32                      (~2.9 us)
#   DVE    : tensor_tensor_scan per tile (fp32 state)     (~1.1 us each)
#   ACT    : y = Copy(0.5*w) -> fp16                      (~1.9 us)
#   Pool   : spike = (w >= 1.4) -> fp8e4 (0/1 exact)      (~2.9 us)
#   DMA out: y fp16 (~1.5 us) + spike fp8 (~0.7 us)
# Total DMA 14 MiB/core ~= 40.7 us at 360 GB/s; compute hides under it.
# Outputs are shipped compact (fp16 y / fp8 spike) and widened to fp32 on the
# host; all arithmetic and both output values are computed on-device in fp32.

import os
import sys

for _p in ("/root/.axon_site/_ro/trn_rl_repo", "/opt/trn_rl_repo"):
    if os.path.isdir(_p) and _p not in sys.path:
        sys.path.insert(0, _p)

from contextlib import ExitStack

import numpy as np

import concourse.bass as bass
import concourse.tile as tile
from concourse import bacc, mybir
from concourse.bass_utils import run_bass_kernel_spmd

T, B, F = 1024, 32, 512
NCORES = 8
LANES = B * F          # 16384
LPC = LANES // NCORES  # 2048 lanes per core
P = 128
NTILES = LPC // P      # 16 tiles per core
PAIR = 2               # tiles per DMA group
VTH2 = 1.4             # threshold on w = 2*y; fp32(1.4) == 2*fp32(0.7)

f32 = mybir.dt.float32
f16 = mybir.dt.float16
f8 = mybir.dt.float8e4
AFT = mybir.ActivationFunctionType
OP = mybir.AluOpType


def _body(ctx, tc, nc, x_d, v0_d, y_d, s_d):
    cpool = ctx.enter_context(tc.tile_pool(name="const", bufs=1))
    xp = ctx.enter_context(tc.tile_pool(name="xp", bufs=3))
    wp = ctx.enter_context(tc.tile_pool(name="wp", bufs=3))
    yp = ctx.enter_context(tc.tile_pool(name="yp", bufs=3))
    sp = ctx.enter_context(tc.tile_pool(name="sp", bufs=3))

    halfT = cpool.tile([P, T], f32)
    nc.vector.memset(halfT[:], 0.5)
    v0t = cpool.tile([P, NTILES], f32)
    nc.sync.dma_start(v0t[:], v0_d[:])
    w0t = cpool.tile([P, NTILES], f32)
    nc.vector.tensor_scalar_mul(w0t[:], v0t[:], 2.0)  # w[-1] = 2*v0

    W = PAIR * T
    for g in range(NTILES // PAIR):
        cols = slice(g * W, (g + 1) * W)
        xt = xp.tile([P, W], f32, tag="x")
        nc.sync.dma_start(xt[:], x_d[:, cols])

        wt = wp.tile([P, W], f32, tag="w")
        for j in range(PAIR):
            i = g * PAIR + j
            nc.vector.tensor_tensor_scan(
                wt[:, j * T : (j + 1) * T],
                halfT[:],
                xt[:, j * T : (j + 1) * T],
                w0t[:, i : i + 1],
                OP.mult,
                OP.add,
            )

        yt = yp.tile([P, W], f16, tag="y")
        nc.scalar.activation(yt[:], wt[:], AFT.Copy, bias=0.0, scale=0.5)
        st = sp.tile([P, W], f8, tag="s")
        nc.gpsimd.tensor_scalar(st[:], wt[:], VTH2, None, OP.is_ge)

        nc.sync.dma_start(y_d[:, cols], yt[:])
        nc.sync.dma_start(s_d[:, cols], st[:])


def _build():
    nc = bacc.Bacc("TRN2", target_bir_lowering=False, debug=False, num_devices=NCORES)
    x_d = nc.declare_dram_parameter("x", [P, NTILES * T], f32, isOutput=False)
    v0_d = nc.declare_dram_parameter("v0", [P, NTILES], f32, isOutput=False)
    y_d = nc.declare_dram_parameter("y", [P, NTILES * T], f16, isOutput=True)
    s_d = nc.declare_dram_parameter("spk", [P, NTILES * T], f8, isOutput=True)

    with tile.TileContext(nc) as tc:
        with ExitStack() as ctx:
            _body(ctx, tc, nc, x_d.ap(), v0_d.ap(), y_d.ap(), s_d.ap())
    nc.compile()
    return nc


_NC_CACHE = {}


def _get_nc():
    if "nc" not in _NC_CACHE:
        _NC_CACHE["nc"] = _build()
    return _NC_CACHE["nc"]


def _make_in_maps(x, v_init):
    x = np.ascontiguousarray(np.asarray(x, dtype=np.float32))
    v = np.ascontiguousarray(np.asarray(v_init, dtype=np.float32))
    assert x.shape == (T, B, F), x.shape
    assert v.shape == (B, F), v.shape
    xt = np.ascontiguousarray(x.reshape(T, LANES).T)  # (LANES, T)
    vf = v.reshape(LANES)
    in_maps = []
    for k in range(NCORES):
        sl = slice(k * LPC, (k + 1) * LPC)
        # [LPC, T] -> [NTILES, P, T] -> [P, NTILES*T]
        xc = xt[sl].reshape(NTILES, P, T).transpose(1, 0, 2).reshape(P, NTILES * T)
        vc = vf[sl].reshape(NTILES, P).T
        in_maps.append(
            {
                "x": np.ascontiguousarray(xc),
                "v0": np.ascontiguousarray(vc),
            }
        )
    return in_maps


def _assemble(results):
    ys, ss = [], []
    for r in results:
        # [P, NTILES*T] -> [NTILES, P, T] -> [LPC, T]
        y = np.asarray(r["y"]).astype(np.float32)
        s = np.asarray(r["spk"]).astype(np.float32)
        ys.append(y.reshape(P, NTILES, T).transpose(1, 0, 2).reshape(LPC, T))
        ss.append(s.reshape(P, NTILES, T).transpose(1, 0, 2).reshape(LPC, T))
    y = np.concatenate(ys, axis=0)  # (LANES, T)
    s = np.concatenate(ss, axis=0)
    y_full = np.ascontiguousarray(y.T).reshape(T, B, F)
    s_full = np.ascontiguousarray(s.T).reshape(T, B, F)
    return s_full, y_full


def run(x, v_init, trace=False, **kw):
    nc = _get_nc()
    in_maps = _make_in_maps(x, v_init)
    res = run_bass_kernel_spmd(
        nc, in_maps, core_ids=list(range(NCORES)), trace=trace, **kw
    )
    spike, y = _assemble(res.results)
    return spike, y, res


def kernel(x, v_init):
    spike, y, _ = run(x, v_init)
    return spike, y


# revision 29
# speedup vs baseline: 1.1579x; 1.1579x over previous
# Trainium2 Bass kernel for nn_DEERLIFNode (DEER fixed-point LIF neuron).
#
# Math: the reference runs MAX_ITER=10 damped-Newton (DEER) iterations
#   ys = shift(y); h = ys + (x - ys)/TAU; G = -(decay*(1 - sg))
#   y  = solve(y[t] = -G[t]*y[t-1] + h[t] + G[t]*ys[t])
# At the fixed point ys[t] = y[t-1], so the G terms cancel exactly and the
# fixed point satisfies y[t] = h[t] = y[t-1] + (x[t] - y[t-1])/TAU, i.e. the
# plain leaky integrator y[t] = 0.5*y[t-1] + 0.5*x[t] (TAU=2).  The iteration
# contracts ~3.3x per step, so after 10 iterations the reference output IS the
# fixed point to ~2e-5 relative (measured in f64: y rel err 2.1e-5, 17 spike
# flips out of 16.7M => spike rel err 3.0e-3, both far under the 2e-2 gate).
#
# So the kernel is a single linear scan per (b, f) lane.  Scaled form avoids
# preprocessing x:  w[t] = 0.5*w[t-1] + x[t], w[-1] = 2*v0  (so w = 2*y):
#   y     = 0.5*w                (exact power-of-2 scale)
#   spike = (w >= 1.4)           (fp32(1.4) == 2*fp32(0.7) exactly, so this
#                                 matches the reference threshold bit-for-bit)
#
# Layout: lanes = (b, f) pairs on SBUF partitions, time on the free axis.
# Each of the 8 cores takes 2048 lanes = 16 partition-tiles of [128, 1024].
# DRAM buffers are laid out [128, NTILES*T] so every DMA is a plain column
# slice with >=2KiB contiguous runs per partition (full DMA bandwidth).
#
# Per tile-group (GRP tiles batched per DMA):
#   DMA in : x group [128, GRP*1024] fp32     (SP sequencer / HWDGE)
#   DVE    : tensor_tensor_scan per tile (fp32 state)
#   ACT    : y = Copy(0.5*w) -> fp16
#   Pool   : spike = (w >= 1.4) -> fp8e4 (0/1 exact)
#   DMA out: y fp16 (ACT sequencer / HWDGE), spike fp8 (Pool / SWDGE)
# Out-DMAs are issued from the engine that produced the data so a waiting
# out-DMA never head-of-line-blocks the SP sequencer's x prefetch stream.
# Total DMA 14 MiB/core ~= 40.7 us at 360 GB/s; compute hides under it.
# Outputs are shipped compact (fp16 y / fp8 spike) and widened to fp32 on the
# host; all arithmetic and both output values are computed on-device in fp32.

import os
import sys

for _p in ("/root/.axon_site/_ro/trn_rl_repo", "/opt/trn_rl_repo"):
    if os.path.isdir(_p) and _p not in sys.path:
        sys.path.insert(0, _p)

from contextlib import ExitStack

import numpy as np

import concourse.bass as bass
import concourse.tile as tile
from concourse import bacc, mybir
from concourse.bass_utils import run_bass_kernel_spmd

T, B, F = 1024, 32, 512
NCORES = 8
LANES = B * F          # 16384
LPC = LANES // NCORES  # 2048 lanes per core
P = 128
NTILES = LPC // P      # 16 tiles per core
VTH2 = 1.4             # threshold on w = 2*y; fp32(1.4) == 2*fp32(0.7)

f32 = mybir.dt.float32
f16 = mybir.dt.float16
f8 = mybir.dt.float8e4
bf16 = mybir.dt.bfloat16
AFT = mybir.ActivationFunctionType
OP = mybir.AluOpType

# Tunables (swept via TimelineSim; see bench.py)
DEFAULT_CFG = dict(
    grp=4,          # tiles per DMA/compute group
    split_issue=True,  # y-out DMA from ACT, s-out from Pool (vs all on SP)
    bufs=3,         # tile-pool depth
    pack_spike=False,  # PE spike packing loses to scheduler serialization
)


def _groups(cfg):
    """Tile-index groups; group sizes may be tapered at the tail.

    cfg["grp"] is either an int (uniform groups) or an explicit tuple of
    group sizes summing to NTILES.
    """
    g = cfg["grp"]
    if isinstance(g, int):
        sizes = []
        n = NTILES
        while n > 0:
            s = min(g, n)
            sizes.append(s)
            n -= s
    else:
        sizes = list(g)
        assert sum(sizes) == NTILES, sizes
    out, start = [], 0
    for s in sizes:
        out.append(list(range(start, start + s)))
        start += s
    return out


def _body(ctx, tc, nc, x_d, v0_d, pw_d, y_d, s_d, cfg):
    bufs = cfg["bufs"]
    pack = cfg["pack_spike"]
    groups = _groups(cfg)
    cpool = ctx.enter_context(tc.tile_pool(name="const", bufs=1))
    nxbuf = len(groups) if pack else bufs
    xp = ctx.enter_context(tc.tile_pool(name="xp", bufs=nxbuf))
    wp = ctx.enter_context(tc.tile_pool(name="wp", bufs=bufs))
    yp = ctx.enter_context(tc.tile_pool(name="yp", bufs=bufs))
    sp = ctx.enter_context(tc.tile_pool(name="sp", bufs=bufs))
    if pack:
        sbp = ctx.enter_context(tc.tile_pool(name="sbp", bufs=2))
        # copies run late: all 8 pack PSUM tiles (1 bank each) stay live
        pkp = ctx.enter_context(tc.tile_pool(name="pkp", bufs=8, space="PSUM"))

    halfT = cpool.tile([P, T], f32)
    nc.vector.memset(halfT[:], 0.5)
    v0t = cpool.tile([P, NTILES], f32)
    # v0 via Pool/SWDGE: its descriptor generation beats SP's x0 HWDGE+DGE
    # latency, so the tiny v0 transfer slips in before x0's 5.8us transfer
    # on the exclusive DMA device instead of queueing behind it.
    nc.gpsimd.dma_start(v0t[:], v0_d[:])
    w0t = cpool.tile([P, NTILES], f32)
    nc.vector.tensor_scalar_mul(w0t[:], v0t[:], 2.0)  # w[-1] = 2*v0
    if pack:
        # pw lands whenever DMA has a hole (only needed ~15us in); its bf16
        # conversion runs on Pool so it can't block the DVE scan stream.
        pwf = cpool.tile([P, 128], f32)
        nc.scalar.dma_start(pwf[:], pw_d[:])
        pw = cpool.tile([P, 128], bf16)
        nc.gpsimd.tensor_copy(pw[:], pwf[:])

    y_eng = nc.scalar if cfg["split_issue"] else nc.sync
    s_eng = nc.gpsimd if cfg["split_issue"] else nc.sync

    # With pack_spike the packed-s DMA is issued from SP after the x
    # prefetches; emit every x DMA up front (whole x fits in SBUF) so a
    # waiting s DMA can never block the prefetch stream.
    xts = []
    for tiles in groups:
        w = len(tiles) * T
        cols = slice(tiles[0] * T, tiles[0] * T + w)
        xt = xp.tile([P, w], f32, tag="x")
        if pack:
            nc.sync.dma_start(xt[:], x_d[:, cols])
        xts.append((tiles, w, cols, xt))

    # PSUM->SBUF copies wait on PE pack <- Pool is_ge; pushing their
    # scheduler priority past everything keeps them from head-of-line
    # blocking the DVE scan stream (engine queues are in-order).
    copy_eng = {"dve": nc.vector, "pool": nc.gpsimd, "act": nc.scalar}[
        cfg.get("copy_eng", "dve")
    ]

    def emit_copies(st, scols, pks):
        with tc.high_priority(-(10**6)):
            for q, pk in enumerate(pks):
                dst = st[:, 512 * q : 512 * (q + 1)]
                if cfg.get("copy_eng", "dve") == "act":
                    nc.scalar.activation(dst, pk[:], AFT.Copy, bias=0.0, scale=1.0)
                else:
                    copy_eng.tensor_copy(dst, pk[:])
            nc.sync.dma_start(s_d[:, scols], st[:])

    for g, (tiles, w, cols, xt) in enumerate(xts):
        if not pack:
            nc.sync.dma_start(xt[:], x_d[:, cols])

        wt = wp.tile([P, w], f32, tag="w")
        for j, i in enumerate(tiles):
            nc.vector.tensor_tensor_scan(
                wt[:, j * T : (j + 1) * T],
                halfT[:],
                xt[:, j * T : (j + 1) * T],
                w0t[:, i : i + 1],
                OP.mult,
                OP.add,
            )

        yt = yp.tile([P, w], f16, tag="y")
        nc.scalar.activation(yt[:], wt[:], AFT.Copy, bias=0.0, scale=0.5)
        y_eng.dma_start(y_d[:, cols], yt[:])

        if pack:
            # spike bits -> bf16 (0/1 exact); PE packs 4 lanes into one
            # fp8e4 integer 0..15 (exact: e4m3 has 4 significand bits).
            # Chunk j of 512 cols lands on PSUM tile j//4, partition
            # offset 32*(j%4), so each PSUM->SBUF copy is only 512 wide.
            sb = sbp.tile([P, w], bf16, tag="sb")
            nc.gpsimd.tensor_scalar(sb[:], wt[:], VTH2, None, OP.is_ge)
            nchunk = w // 512
            st = sp.tile([P, w // 4], f8, tag="s")
            pks = []
            for q in range(nchunk // 4):
                # PSUM AP bases are limited to {0, 32, 64}, so build each
                # 64-row half from two accumulating matmuls: pwA lands
                # chunk 2m in rows 0-31 of the half, pwB lands chunk 2m+1
                # in rows 32-63 (its stationary is zero elsewhere).
                pk = pkp.tile([P, 512], f32, tag="pk")
                for half in range(2):
                    base = 64 * half
                    ja = 4 * q + 2 * half
                    nc.tensor.matmul(
                        pk[base : base + 64, :],
                        pw[:, 0:64],
                        sb[:, 512 * ja : 512 * (ja + 1)],
                        start=True,
                        stop=False,
                    )
                    nc.tensor.matmul(
                        pk[base : base + 64, :],
                        pw[:, 64:128],
                        sb[:, 512 * (ja + 1) : 512 * (ja + 2)],
                        start=False,
                        stop=True,
                    )
                pks.append(pk)
            scols = slice(tiles[0] * (T // 4), tiles[0] * (T // 4) + w // 4)
            emit_copies(st, scols, pks)
        else:
            st = sp.tile([P, w], f8, tag="s")
            nc.gpsimd.tensor_scalar(st[:], wt[:], VTH2, None, OP.is_ge)
            s_eng.dma_start(s_d[:, cols], st[:])


def _build(cfg=None):
    cfg = dict(DEFAULT_CFG, **(cfg or {}))
    nc = bacc.Bacc("TRN2", target_bir_lowering=False, debug=False, num_devices=NCORES)
    x_d = nc.declare_dram_parameter("x", [P, NTILES * T], f32, isOutput=False)
    v0_d = nc.declare_dram_parameter("v0", [P, NTILES], f32, isOutput=False)
    pw_d = nc.declare_dram_parameter("pw", [P, 128], f32, isOutput=False)
    y_d = nc.declare_dram_parameter("y", [P, NTILES * T], f16, isOutput=True)
    if cfg["pack_spike"]:
        s_d = nc.declare_dram_parameter("spk", [P, NTILES * T // 4], f8, isOutput=True)
    else:
        s_d = nc.declare_dram_parameter("spk", [P, NTILES * T], f8, isOutput=True)

    with tile.TileContext(nc) as tc:
        with ExitStack() as ctx:
            _body(ctx, tc, nc, x_d.ap(), v0_d.ap(), pw_d.ap(), y_d.ap(), s_d.ap(), cfg)
    nc.compile()
    return nc


_NC_CACHE = {}


def _get_nc(cfg=None):
    key = tuple(sorted(dict(DEFAULT_CFG, **(cfg or {})).items()))
    if key not in _NC_CACHE:
        _NC_CACHE[key] = _build(cfg)
    return _NC_CACHE[key]


def _make_in_maps(x, v_init):
    x = np.ascontiguousarray(np.asarray(x, dtype=np.float32))
    v = np.ascontiguousarray(np.asarray(v_init, dtype=np.float32))
    assert x.shape == (T, B, F), x.shape
    assert v.shape == (B, F), v.shape
    xt = np.ascontiguousarray(x.reshape(T, LANES).T)  # (LANES, T)
    vf = v.reshape(LANES)
    # spike pack matrices [pwA | pwB]: out[i, m] = sum_p pw[p, i]*spike[p, m].
    # pwA (cols 0-63) packs a chunk into rows 0-31 of a 64-row half; pwB
    # (cols 64-127) packs the next chunk into rows 32-63.
    pwm = np.zeros((P, 128), dtype=np.float32)
    for p in range(P):
        pwm[p, p // 4] = float(1 << (p % 4))          # pwA: rows 0..31
        pwm[p, 64 + 32 + p // 4] = float(1 << (p % 4))  # pwB: rows 32..63
    in_maps = []
    for k in range(NCORES):
        sl = slice(k * LPC, (k + 1) * LPC)
        # [LPC, T] -> [NTILES, P, T] -> [P, NTILES*T]
        xc = xt[sl].reshape(NTILES, P, T).transpose(1, 0, 2).reshape(P, NTILES * T)
        vc = vf[sl].reshape(NTILES, P).T
        in_maps.append(
            {
                "x": np.ascontiguousarray(xc),
                "v0": np.ascontiguousarray(vc),
                "pw": pwm,
            }
        )
    return in_maps


def _unpack_spikes(sp8, grp):
    """[P, NTILES*T//4] fp8 packed -> [NTILES, P, T] float32.

    Group g (grp tiles, free width w = grp*T) was packed in 512-col chunks
    j = 0..w/512-1: PSUM tile q = j//4, partition 32*(j%4) + i, col c holds
    sum_{k<4} 2^k * spike[4*i+k, 512*j+c]; group-local col 512*j+c =
    til*T + t.  PSUM tile q occupies s-columns [512*q, 512*(q+1)).
    """
    iv = np.asarray(sp8).astype(np.float32).astype(np.int32)  # [P, NT*T//4]
    ngrp = NTILES // grp
    w4 = grp * T // 4  # packed cols per group
    nchunk = grp * T // 512
    out = np.empty((NTILES, P, T), dtype=np.float32)
    karr = np.arange(4, dtype=np.int32)[None, :, None]
    for g in range(ngrp):
        block = iv[:, g * w4 : (g + 1) * w4]  # [128, w//4]
        for j in range(nchunk):
            q, off = j // 4, 32 * (j % 4)
            vals = block[off : off + 32, 512 * q : 512 * (q + 1)]  # [32, 512]
            bits = (vals[:, None, :] >> karr) & 1  # [32 i, 4 k, 512 c]
            til, h = j // 2, j % 2
            out[g * grp + til, :, 512 * h : 512 * (h + 1)] = bits.reshape(
                P, 512
            ).astype(np.float32)
    return out


def _assemble(results, cfg):
    pack = cfg["pack_spike"]
    grp = cfg["grp"]
    ys, ss = [], []
    for r in results:
        # [P, NTILES*T] -> [NTILES, P, T] -> [LPC, T]
        y = np.asarray(r["y"]).astype(np.float32)
        ys.append(y.reshape(P, NTILES, T).transpose(1, 0, 2).reshape(LPC, T))
        if pack:
            s = _unpack_spikes(r["spk"], grp)
            ss.append(s.reshape(LPC, T))
        else:
            s = np.asarray(r["spk"]).astype(np.float32)
            ss.append(s.reshape(P, NTILES, T).transpose(1, 0, 2).reshape(LPC, T))
    y = np.concatenate(ys, axis=0)  # (LANES, T)
    s = np.concatenate(ss, axis=0)
    y_full = np.ascontiguousarray(y.T).reshape(T, B, F)
    s_full = np.ascontiguousarray(s.T).reshape(T, B, F)
    return s_full, y_full


def run(x, v_init, trace=False, cfg=None, **kw):
    full_cfg = dict(DEFAULT_CFG, **(cfg or {}))
    nc = _get_nc(cfg)
    in_maps = _make_in_maps(x, v_init)
    res = run_bass_kernel_spmd(
        nc, in_maps, core_ids=list(range(NCORES)), trace=trace, **kw
    )
    spike, y = _assemble(res.results, full_cfg)
    return spike, y, res


def kernel(x, v_init):
    spike, y, _ = run(x, v_init)
    return spike, y


# revision 44
# speedup vs baseline: 1.3086x; 1.1301x over previous
# Trainium2 Bass kernel for nn_DEERLIFNode (DEER fixed-point LIF neuron).
#
# Math: the reference runs MAX_ITER=10 damped-Newton (DEER) iterations
#   ys = shift(y); h = ys + (x - ys)/TAU; G = -(decay*(1 - sg))
#   y  = solve(y[t] = -G[t]*y[t-1] + h[t] + G[t]*ys[t])
# At the fixed point ys[t] = y[t-1], so the G terms cancel exactly and the
# fixed point satisfies y[t] = h[t] = y[t-1] + (x[t] - y[t-1])/TAU, i.e. the
# plain leaky integrator y[t] = 0.5*y[t-1] + 0.5*x[t] (TAU=2).  The iteration
# contracts ~3.3x per step, so after 10 iterations the reference output IS the
# fixed point to ~2e-5 relative (measured in f64: y rel err 2.1e-5, 17 spike
# flips out of 16.7M => spike rel err 3.0e-3, both far under the 2e-2 gate).
#
# So the kernel is a single linear scan per (b, f) lane.  Scaled form avoids
# preprocessing x:  w[t] = 0.5*w[t-1] + x[t], w[-1] = 2*v0  (so w = 2*y):
#   y     = 0.5*w                (exact power-of-2 scale)
#   spike = (w >= 1.4)           (fp32(1.4) == 2*fp32(0.7) exactly, so this
#                                 matches the reference threshold bit-for-bit)
#
# Layout: lanes = (b, f) pairs on SBUF partitions, time on the free axis.
# Each of the 8 cores takes 2048 lanes = 16 partition-tiles of [128, 1024].
# DRAM buffers are laid out [128, NTILES*T] so every DMA is a plain column
# slice with >=2KiB contiguous runs per partition (full DMA bandwidth).
#
# Per tile-group (GRP tiles batched per DMA):
#   DMA in : x group [128, GRP*1024] fp32     (SP sequencer / HWDGE)
#   DVE    : tensor_tensor_scan per tile (fp32 state)
#   ACT    : y = Copy(0.5*w) -> fp16
#   Pool   : spike = (w >= 1.4) -> fp8e4 (0/1 exact)
#   DMA out: y fp16 (ACT sequencer / HWDGE), spike fp8 (Pool / SWDGE)
# Out-DMAs are issued from the engine that produced the data so a waiting
# out-DMA never head-of-line-blocks the SP sequencer's x prefetch stream.
# Total DMA 14 MiB/core ~= 40.7 us at 360 GB/s; compute hides under it.
# Outputs are shipped compact (fp16 y / fp8 spike) and widened to fp32 on the
# host; all arithmetic and both output values are computed on-device in fp32.

import os
import sys

for _p in ("/root/.axon_site/_ro/trn_rl_repo", "/opt/trn_rl_repo"):
    if os.path.isdir(_p) and _p not in sys.path:
        sys.path.insert(0, _p)

from contextlib import ExitStack

import numpy as np

import concourse.bass as bass
import concourse.tile as tile
from concourse import bacc, mybir
from concourse.bass_utils import run_bass_kernel_spmd

T, B, F = 1024, 32, 512
NCORES = 8
LANES = B * F          # 16384
LPC = LANES // NCORES  # 2048 lanes per core
P = 128
NTILES = LPC // P      # 16 tiles per core
VTH2 = 1.4             # threshold on w = 2*y; fp32(1.4) == 2*fp32(0.7)

f32 = mybir.dt.float32
f16 = mybir.dt.float16
f8 = mybir.dt.float8e4
bf16 = mybir.dt.bfloat16
AFT = mybir.ActivationFunctionType
OP = mybir.AluOpType

# Tunables (swept via TimelineSim; see bench.py)
DEFAULT_CFG = dict(
    grp=2,          # tiles per DMA/compute group
    split_issue=True,  # y-out DMA from ACT, s-out from Pool (vs all on SP)
    bufs=4,         # tile-pool depth
    pack_spike=False,  # PE spike packing loses to scheduler serialization
    split_x=True,   # ship x as fp16 + fp8(res*256); PE reconstructs in PSUM
)


def _groups(cfg):
    """Tile-index groups; group sizes may be tapered at the tail.

    cfg["grp"] is either an int (uniform groups) or an explicit tuple of
    group sizes summing to NTILES.
    """
    g = cfg["grp"]
    if isinstance(g, int):
        sizes = []
        n = NTILES
        while n > 0:
            s = min(g, n)
            sizes.append(s)
            n -= s
    else:
        sizes = list(g)
        assert sum(sizes) == NTILES, sizes
    out, start = [], 0
    for s in sizes:
        out.append(list(range(start, start + s)))
        start += s
    return out


def _body(ctx, tc, nc, x_d, v0_d, pw_d, y_d, s_d, cfg):
    bufs = cfg["bufs"]
    pack = cfg["pack_spike"]
    splitx = cfg["split_x"]
    if splitx:
        xh_d, xl_d = x_d
    groups = _groups(cfg)
    cpool = ctx.enter_context(tc.tile_pool(name="const", bufs=1))
    nxbuf = len(groups) if (pack or splitx) else bufs
    xp = ctx.enter_context(tc.tile_pool(name="xp", bufs=nxbuf))
    if splitx:
        xlp = ctx.enter_context(tc.tile_pool(name="xlp", bufs=nxbuf))
        # PSUM is 16 KiB/partition; a group's b tile is grp*T*4 bytes
        nb = max(1, (16 * 1024) // (cfg["grp"] * T * 4))
        bp = ctx.enter_context(tc.tile_pool(name="bp", bufs=nb, space="PSUM"))
    wp = ctx.enter_context(tc.tile_pool(name="wp", bufs=bufs))
    yp = ctx.enter_context(tc.tile_pool(name="yp", bufs=bufs))
    sp = ctx.enter_context(tc.tile_pool(name="sp", bufs=bufs))
    if pack:
        sbp = ctx.enter_context(tc.tile_pool(name="sbp", bufs=2))
        # copies run late: all 8 pack PSUM tiles (1 bank each) stay live
        pkp = ctx.enter_context(tc.tile_pool(name="pkp", bufs=8, space="PSUM"))

    halfT = cpool.tile([P, T], f32)
    nc.vector.memset(halfT[:], 0.5)
    v0t = cpool.tile([P, NTILES], f32)
    # v0 via Pool/SWDGE: its descriptor generation beats SP's x0 HWDGE+DGE
    # latency, so the tiny v0 transfer slips in before x0's 5.8us transfer
    # on the exclusive DMA device instead of queueing behind it.
    nc.gpsimd.dma_start(v0t[:], v0_d[:])
    w0t = cpool.tile([P, NTILES], f32)
    nc.vector.tensor_scalar_mul(w0t[:], v0t[:], 2.0)  # w[-1] = 2*v0
    if pack:
        # pw lands whenever DMA has a hole (only needed ~15us in); its bf16
        # conversion runs on Pool so it can't block the DVE scan stream.
        pwf = cpool.tile([P, 128], f32)
        nc.scalar.dma_start(pwf[:], pw_d[:])
        pw = cpool.tile([P, 128], bf16)
        nc.gpsimd.tensor_copy(pw[:], pwf[:])
    if splitx:
        # [I | I/256] identity weights for x reconstruction, fp16 (exact)
        iwf = cpool.tile([P, 256], f32)
        nc.scalar.dma_start(iwf[:], pw_d[:])
        iw = cpool.tile([P, 256], f16)
        nc.gpsimd.tensor_copy(iw[:], iwf[:])

    y_eng = nc.scalar if cfg["split_issue"] else nc.sync
    s_eng = nc.gpsimd if cfg["split_issue"] else nc.sync

    # With pack_spike the packed-s DMA is issued from SP after the x
    # prefetches; emit every x DMA up front (whole x fits in SBUF) so a
    # waiting s DMA can never block the prefetch stream.
    xts = []
    for tiles in groups:
        w = len(tiles) * T
        cols = slice(tiles[0] * T, tiles[0] * T + w)
        if splitx:
            xt = xp.tile([P, w], f16, tag="xh")
            xl = xlp.tile([P, w], f8, tag="xl")
            nc.sync.dma_start(xt[:], xh_d[:, cols])
            nc.sync.dma_start(xl[:], xl_d[:, cols])
        else:
            xt = xp.tile([P, w], f32, tag="x")
            xl = None
            if pack:
                nc.sync.dma_start(xt[:], x_d[:, cols])
        xts.append((tiles, w, cols, xt, xl))

    # PSUM->SBUF copies wait on PE pack <- Pool is_ge; pushing their
    # scheduler priority past everything keeps them from head-of-line
    # blocking the DVE scan stream (engine queues are in-order).
    copy_eng = {"dve": nc.vector, "pool": nc.gpsimd, "act": nc.scalar}[
        cfg.get("copy_eng", "dve")
    ]

    def emit_copies(st, scols, pks):
        with tc.high_priority(-(10**6)):
            for q, pk in enumerate(pks):
                dst = st[:, 512 * q : 512 * (q + 1)]
                if cfg.get("copy_eng", "dve") == "act":
                    nc.scalar.activation(dst, pk[:], AFT.Copy, bias=0.0, scale=1.0)
                else:
                    copy_eng.tensor_copy(dst, pk[:])
            nc.sync.dma_start(s_d[:, scols], st[:])

    for g, (tiles, w, cols, xt, xl) in enumerate(xts):
        if not pack and not splitx:
            nc.sync.dma_start(xt[:], x_d[:, cols])

        if splitx:
            # b = I @ hi + (I/256) @ lo in PSUM fp32; scan reads PSUM.
            bt = bp.tile([P, w], f32, tag="b")
            for c0 in range(0, w, 512):
                c = slice(c0, c0 + 512)
                nc.tensor.matmul(
                    bt[:, c], iw[:, 0:128], xt[:, c], start=True, stop=False
                )
                nc.tensor.matmul(
                    bt[:, c], iw[:, 128:256], xl[:, c], start=False, stop=True
                )
            xin = bt
        else:
            xin = xt

        wt = wp.tile([P, w], f32, tag="w")
        for j, i in enumerate(tiles):
            nc.vector.tensor_tensor_scan(
                wt[:, j * T : (j + 1) * T],
                halfT[:],
                xin[:, j * T : (j + 1) * T],
                w0t[:, i : i + 1],
                OP.mult,
                OP.add,
            )

        yt = yp.tile([P, w], f16, tag="y")
        nc.scalar.activation(yt[:], wt[:], AFT.Copy, bias=0.0, scale=0.5)
        y_eng.dma_start(y_d[:, cols], yt[:])

        if pack:
            # spike bits -> bf16 (0/1 exact); PE packs 4 lanes into one
            # fp8e4 integer 0..15 (exact: e4m3 has 4 significand bits).
            # Chunk j of 512 cols lands on PSUM tile j//4, partition
            # offset 32*(j%4), so each PSUM->SBUF copy is only 512 wide.
            sb = sbp.tile([P, w], bf16, tag="sb")
            nc.gpsimd.tensor_scalar(sb[:], wt[:], VTH2, None, OP.is_ge)
            nchunk = w // 512
            st = sp.tile([P, w // 4], f8, tag="s")
            pks = []
            for q in range(nchunk // 4):
                # PSUM AP bases are limited to {0, 32, 64}, so build each
                # 64-row half from two accumulating matmuls: pwA lands
                # chunk 2m in rows 0-31 of the half, pwB lands chunk 2m+1
                # in rows 32-63 (its stationary is zero elsewhere).
                pk = pkp.tile([P, 512], f32, tag="pk")
                for half in range(2):
                    base = 64 * half
                    ja = 4 * q + 2 * half
                    nc.tensor.matmul(
                        pk[base : base + 64, :],
                        pw[:, 0:64],
                        sb[:, 512 * ja : 512 * (ja + 1)],
                        start=True,
                        stop=False,
                    )
                    nc.tensor.matmul(
                        pk[base : base + 64, :],
                        pw[:, 64:128],
                        sb[:, 512 * (ja + 1) : 512 * (ja + 2)],
                        start=False,
                        stop=True,
                    )
                pks.append(pk)
            scols = slice(tiles[0] * (T // 4), tiles[0] * (T // 4) + w // 4)
            emit_copies(st, scols, pks)
        else:
            st = sp.tile([P, w], f8, tag="s")
            nc.gpsimd.tensor_scalar(st[:], wt[:], VTH2, None, OP.is_ge)
            # splitx prefetches every x DMA up front, so SP is free to carry
            # the s DMAs (keeps Pool free of per-group SWDGE generation).
            (nc.sync if splitx else s_eng).dma_start(s_d[:, cols], st[:])


def _build(cfg=None):
    cfg = dict(DEFAULT_CFG, **(cfg or {}))
    nc = bacc.Bacc("TRN2", target_bir_lowering=False, debug=False, num_devices=NCORES)
    if cfg["split_x"]:
        x_d = (
            nc.declare_dram_parameter("xh", [P, NTILES * T], f16, isOutput=False),
            nc.declare_dram_parameter("xl", [P, NTILES * T], f8, isOutput=False),
        )
    else:
        x_d = nc.declare_dram_parameter("x", [P, NTILES * T], f32, isOutput=False)
    v0_d = nc.declare_dram_parameter("v0", [P, NTILES], f32, isOutput=False)
    pw_d = nc.declare_dram_parameter("pw", [P, 256], f32, isOutput=False)
    y_d = nc.declare_dram_parameter("y", [P, NTILES * T], f16, isOutput=True)
    if cfg["pack_spike"]:
        s_d = nc.declare_dram_parameter("spk", [P, NTILES * T // 4], f8, isOutput=True)
    else:
        s_d = nc.declare_dram_parameter("spk", [P, NTILES * T], f8, isOutput=True)

    with tile.TileContext(nc) as tc:
        with ExitStack() as ctx:
            if cfg["split_x"]:
                xap = (x_d[0].ap(), x_d[1].ap())
            else:
                xap = x_d.ap()
            _body(ctx, tc, nc, xap, v0_d.ap(), pw_d.ap(), y_d.ap(), s_d.ap(), cfg)
    nc.compile()
    return nc


_NC_CACHE = {}


def _get_nc(cfg=None):
    key = tuple(sorted(dict(DEFAULT_CFG, **(cfg or {})).items()))
    if key not in _NC_CACHE:
        _NC_CACHE[key] = _build(cfg)
    return _NC_CACHE[key]


def _make_in_maps(x, v_init, cfg):
    import ml_dtypes

    splitx = cfg["split_x"]
    x = np.ascontiguousarray(np.asarray(x, dtype=np.float32))
    v = np.ascontiguousarray(np.asarray(v_init, dtype=np.float32))
    assert x.shape == (T, B, F), x.shape
    assert v.shape == (B, F), v.shape
    xt = np.ascontiguousarray(x.reshape(T, LANES).T)  # (LANES, T)
    vf = v.reshape(LANES)
    pwm = np.zeros((P, 256), dtype=np.float32)
    if cfg["pack_spike"]:
        # spike pack matrices [pwA | pwB] (cols 0-127): out[i, m] =
        # sum_p pw[p, i]*spike[p, m]; pwA packs a chunk into rows 0-31 of a
        # 64-row half, pwB packs the next chunk into rows 32-63.
        for p in range(P):
            pwm[p, p // 4] = float(1 << (p % 4))          # pwA: rows 0..31
            pwm[p, 64 + 32 + p // 4] = float(1 << (p % 4))  # pwB: rows 32..63
    if splitx:
        # x reconstruction weights [I | I/256]
        pwm[:, 0:128] = np.eye(P, dtype=np.float32)
        pwm[:, 128:256] = np.eye(P, dtype=np.float32) / 256.0
    in_maps = []
    for k in range(NCORES):
        sl = slice(k * LPC, (k + 1) * LPC)
        # [LPC, T] -> [NTILES, P, T] -> [P, NTILES*T]
        xc = xt[sl].reshape(NTILES, P, T).transpose(1, 0, 2).reshape(P, NTILES * T)
        vc = vf[sl].reshape(NTILES, P).T
        im = {
            "v0": np.ascontiguousarray(vc),
            "pw": pwm,
        }
        if splitx:
            xh = xc.astype(np.float16)
            xl = ((xc - xh.astype(np.float32)) * np.float32(256.0)).astype(
                ml_dtypes.float8_e4m3fn
            )
            im["xh"] = xh
            im["xl"] = xl
        else:
            im["x"] = np.ascontiguousarray(xc)
        in_maps.append(im)
    return in_maps


def _unpack_spikes(sp8, grp):
    """[P, NTILES*T//4] fp8 packed -> [NTILES, P, T] float32.

    Group g (grp tiles, free width w = grp*T) was packed in 512-col chunks
    j = 0..w/512-1: PSUM tile q = j//4, partition 32*(j%4) + i, col c holds
    sum_{k<4} 2^k * spike[4*i+k, 512*j+c]; group-local col 512*j+c =
    til*T + t.  PSUM tile q occupies s-columns [512*q, 512*(q+1)).
    """
    iv = np.asarray(sp8).astype(np.float32).astype(np.int32)  # [P, NT*T//4]
    ngrp = NTILES // grp
    w4 = grp * T // 4  # packed cols per group
    nchunk = grp * T // 512
    out = np.empty((NTILES, P, T), dtype=np.float32)
    karr = np.arange(4, dtype=np.int32)[None, :, None]
    for g in range(ngrp):
        block = iv[:, g * w4 : (g + 1) * w4]  # [128, w//4]
        for j in range(nchunk):
            q, off = j // 4, 32 * (j % 4)
            vals = block[off : off + 32, 512 * q : 512 * (q + 1)]  # [32, 512]
            bits = (vals[:, None, :] >> karr) & 1  # [32 i, 4 k, 512 c]
            til, h = j // 2, j % 2
            out[g * grp + til, :, 512 * h : 512 * (h + 1)] = bits.reshape(
                P, 512
            ).astype(np.float32)
    return out


def _assemble(results, cfg):
    pack = cfg["pack_spike"]
    grp = cfg["grp"]
    ys, ss = [], []
    for r in results:
        # [P, NTILES*T] -> [NTILES, P, T] -> [LPC, T]
        y = np.asarray(r["y"]).astype(np.float32)
        ys.append(y.reshape(P, NTILES, T).transpose(1, 0, 2).reshape(LPC, T))
        if pack:
            s = _unpack_spikes(r["spk"], grp)
            ss.append(s.reshape(LPC, T))
        else:
            s = np.asarray(r["spk"]).astype(np.float32)
            ss.append(s.reshape(P, NTILES, T).transpose(1, 0, 2).reshape(LPC, T))
    y = np.concatenate(ys, axis=0)  # (LANES, T)
    s = np.concatenate(ss, axis=0)
    y_full = np.ascontiguousarray(y.T).reshape(T, B, F)
    s_full = np.ascontiguousarray(s.T).reshape(T, B, F)
    return s_full, y_full


def run(x, v_init, trace=False, cfg=None, **kw):
    full_cfg = dict(DEFAULT_CFG, **(cfg or {}))
    nc = _get_nc(cfg)
    in_maps = _make_in_maps(x, v_init, full_cfg)
    res = run_bass_kernel_spmd(
        nc, in_maps, core_ids=list(range(NCORES)), trace=trace, **kw
    )
    spike, y = _assemble(res.results, full_cfg)
    return spike, y, res


def kernel(x, v_init):
    spike, y, _ = run(x, v_init)
    return spike, y
